# revision 47
# baseline (speedup 1.0000x reference)
"""Multi-head attention (B=2, T=2048, D=512, H=8) on 8 trn2 NeuronCores.

Sharding: data + head parallel (core c: batch c//4, head pair c%4).

v3 (all-bf16 math):
  - q/k projections -> bf16 qT/kT [feat, tok] sbuf.
  - QK per key tile: scoresT psum [128 keys, 2 heads, 512 q] f32.
  - softmax exp is split across TWO engines by key-tile parity: even kt on
    ACT (exact exp), odd kt on DVE via a Schraudolph bit-trick:
    u16 = round_sat(s_raw * 16*log2e + 16250) bitcast to bf16 ~= e^(s/8)
    with a +-3% mantissa-interpolation sawtooth (averages out in softmax;
    measured rel err 1.2e-2 < 2e-2).  Parity matches the 2-buf scores ring
    so both engines stream concurrently instead of in alternating bursts.
  - PV in query-major orientation: stationary = ex slice [keys, 128 q],
    moving = v_aug [keys, 65] (col 64 = ones -> softmax denominator per
    query lands in psum column 64, i.e. per-PARTITION scalar).  Out
    pvq_h [128 q, 65] f32, 65 rows/matmul instead of 512 -> PV drops from
    65536 to 33280 PE rows.
  - normalize: one DVE tensor_tensor divide per (qsub, head) with a
    stride-0 broadcast of the den column; writes att [q, head-feat] bf16.
  - att is transposed [q,128] -> [128,q] by a DMA-engine transpose (XBAR),
    then ONE Wo matmul per qsub contracts all 128 features of both heads
    (Wo drops from 16384 to 8192 PE rows).
  - host sums the 4 partial outputs per batch and adds bo.
"""

import os
import sys

sys.path.insert(0, "/opt/trn_rl_repo")

from contextlib import ExitStack

import numpy as np
import ml_dtypes

import concourse.bass as bass
import concourse.tile as tile
from concourse import bacc, mybir
from concourse.bass_utils import run_bass_kernel_spmd

BF16 = mybir.dt.bfloat16
F32 = mybir.dt.float32
U16 = mybir.dt.uint16

B, T, D = 2, 2048, 512
H, DK = 8, 64
N_CORES = 8
P = 128
KC = D // P  # 4 contraction chunks of 128 over d_model
NKT = T // P  # 16 key tiles of 128
NQB = 4  # query blocks
QB = T // NQB  # 512 queries per block
QSUB = QB // P  # 4 sub-blocks of 128 queries
NTC = 4  # token chunks for pipelined loads/projections
VW = DK + 1  # v_aug width: 64 feats + ones column

LOG2E = 1.4426950408889634
# phase-averaged Schraudolph: ex = S(y-1) + S(y-0.5)*2^-0.5 ~= e^s with the
# log-linear mantissa sawtooth averaged down to +-0.75%
B16_SCH1 = 16250.0 - 128.0
B16_SCH2 = 16250.0 - 128.0 + 64.0
C_SCH = float(2.0 ** -0.5)

CFG = {
    # exp engine per kt (len NKT, per qb): must be constant per kt%2 chain
    # so the 2-buf scores ring keeps both engines streaming.
    "exp_engines": "AAAAADAAAAADAAAA",
    "qk_copy": "DDDDDDDD",  # 8 chunk copies
    "v_copy": "DDDDDDDDDDDDDDDD",  # 16 kt copies
    "out_copy": "DDDDDDDDDDDDAAAA",  # 16 qsub copies (last qb on idle ACT)
    "attT_copy": "DDDDDDDDDDDDDDDD",  # 16 transpose copies
    "exp_bufs": 32,
    "att_bufs": 8,
    "out_bufs": 6,
}


def _build_bass(with_bias):
    nc = bacc.Bacc(trn_type="TRN2", num_devices=N_CORES, debug=False)

    qt_d = nc.dram_tensor("qt", [D, T], BF16, kind="ExternalInput").ap()
    kt_d = nc.dram_tensor("ktin", [D, T], BF16, kind="ExternalInput").ap()
    vt_d = nc.dram_tensor("vt", [D, T], BF16, kind="ExternalInput").ap()
    wqkv_d = nc.dram_tensor("wqkv", [P, 3, KC, P], BF16, kind="ExternalInput").ap()
    wot_d = nc.dram_tensor("wot", [P, D], BF16, kind="ExternalInput").ap()
    if with_bias:
        bq_d = nc.dram_tensor("bq", [P, 1], F32, kind="ExternalInput").ap()
        bk_d = nc.dram_tensor("bk", [P, 1], F32, kind="ExternalInput").ap()
        bv_d = nc.dram_tensor("bv", [1, P], F32, kind="ExternalInput").ap()
    out_d = nc.dram_tensor("outp", [T, D], F32, kind="ExternalOutput").ap()

    with tile.TileContext(nc) as tc, ExitStack() as ctx:
        singles = ctx.enter_context(tc.tile_pool(name="singles", bufs=1))
        exp_pool = ctx.enter_context(tc.tile_pool(name="exps", bufs=CFG["exp_bufs"]))
        att_pool = ctx.enter_context(tc.tile_pool(name="att", bufs=CFG["att_bufs"]))
        out_pool = ctx.enter_context(tc.tile_pool(name="outs", bufs=CFG["out_bufs"]))
        ps_s = ctx.enter_context(tc.tile_pool(name="ps_s", bufs=2, space="PSUM"))
        ps_pv = ctx.enter_context(tc.tile_pool(name="ps_pv", bufs=1, space="PSUM"))
        ps_mi = ctx.enter_context(tc.tile_pool(name="ps_mi", bufs=2, space="PSUM"))
        ps_sd = ctx.enter_context(tc.tile_pool(name="ps_sd", bufs=1, space="PSUM"))
        scr_pool = ctx.enter_context(tc.tile_pool(name="scr", bufs=6))

        # ---- weight/bias loads ----
        wqkv_sb = singles.tile([P, 3, KC, P], BF16)
        nc.sync.dma_start(out=wqkv_sb, in_=wqkv_d)
        wqt_sb = wqkv_sb[:, 0]
        wkt_sb = wqkv_sb[:, 1]
        wvt_sb = wqkv_sb[:, 2]
        if with_bias:
            bq_sb = singles.tile([P, 1], F32)
            nc.sync.dma_start(out=bq_sb, in_=bq_d)
            bk_sb = singles.tile([P, 1], F32)
            nc.sync.dma_start(out=bk_sb, in_=bk_d)
            bv_sb = singles.tile([P, P], F32)
            nc.gpsimd.dma_start(
                out=bv_sb,
                in_=bass.AP(tensor=bv_d.tensor, offset=0, ap=[[0, P], [1, P]]),
            )

        # warm the ACT exp table during the DMA-bound prologue
        warm_in = singles.tile([1, 2], F32)
        nc.vector.memset(warm_in, 0.0)
        warm_out = singles.tile([1, 2], BF16)
        nc.scalar.activation(out=warm_out, in_=warm_in,
                             func=mybir.ActivationFunctionType.Exp, scale=1.0)

        qT = singles.tile([P, T], BF16)
        kT = singles.tile([P, T], BF16)

        # identity matrix for PE transposes
        ident_i = singles.tile([P, P], mybir.dt.int32)
        nc.gpsimd.iota(ident_i, [[0, P]], base=0, channel_multiplier=1)
        ident_j = singles.tile([P, P], mybir.dt.int32)
        nc.gpsimd.iota(ident_j, [[1, P]], base=0, channel_multiplier=0)
        ident_m = singles.tile([P, P], BF16)
        nc.vector.tensor_tensor(out=ident_m, in0=ident_i, in1=ident_j,
                                op=mybir.AluOpType.is_equal)
        ident_sb = ident_m

        # v_aug tiles: [tok, 2 head, 65] bf16, ones in col 64
        v_aug = []
        for kt in range(NKT):
            va = singles.tile([P, 2, VW], BF16, name=f"vaug{kt}")
            nc.gpsimd.memset(va[:, :, DK : DK + 1], 1.0)
            v_aug.append(va)

        # ---- chunked input loads ----
        qt_sb = singles.tile([P, KC, T], BF16)
        kt_sb = singles.tile([P, KC, T], BF16)
        vt_sb = singles.tile([P, KC, T], BF16)
        ktr = kt_d.rearrange("(c p) t -> p c t", p=P)
        qtr = qt_d.rearrange("(c p) t -> p c t", p=P)
        vtr = vt_d.rearrange("(c p) t -> p c t", p=P)
        nc.sync.dma_start(out=kt_sb[:, :, 0:P], in_=ktr[:, :, 0:P])
        nc.scalar.dma_start(out=qt_sb[:, :, 0 : T // NTC // 2], in_=qtr[:, :, 0 : T // NTC // 2])
        nc.scalar.dma_start(
            out=qt_sb[:, :, T // NTC // 2 : T // NTC],
            in_=qtr[:, :, T // NTC // 2 : T // NTC],
        )
        nc.sync.dma_start(out=kt_sb[:, :, P : T // NTC], in_=ktr[:, :, P : T // NTC])
        for c in range(1, NTC):
            sl = bass.ts(c, T // NTC)
            nc.sync.dma_start(out=kt_sb[:, :, sl], in_=ktr[:, :, sl])
            slp = bass.ts(c - 1, T // NTC)
            if c >= 2:
                nc.scalar.dma_start(out=qt_sb[:, :, slp], in_=qtr[:, :, slp])
            nc.gpsimd.dma_start(out=vt_sb[:, :, slp], in_=vtr[:, :, slp])
        slz = bass.ts(NTC - 1, T // NTC)
        nc.scalar.dma_start(out=qt_sb[:, :, slz], in_=qtr[:, :, slz])
        nc.gpsimd.dma_start(out=vt_sb[:, :, slz], in_=vtr[:, :, slz])
        wot_sb = singles.tile([P, D], BF16)
        nc.sync.dma_start(out=wot_sb, in_=wot_d)

        inv_sqrt_dk = float(1.0 / np.sqrt(DK))
        idxs = {"qk_copy": [0], "v_copy": [0], "out_copy": [0], "attT_copy": [0]}

        def eng_of(key):
            box = idxs[key]
            e = CFG[key][box[0] % len(CFG[key])]
            box[0] += 1
            return e

        def emit_copy(e, out, in_):
            if e == "A":
                nc.scalar.copy(out, in_)
            else:
                nc.vector.tensor_copy(out, in_)

        def emit_qk_proj(dst, src_sb, w_sb, b_sb, c, lo=None, cs=None):
            if cs is None:
                cs = T // NTC
            lo = c * (T // NTC) if lo is None else lo
            sl = bass.ds(lo, cs)
            psq = ps_mi.tile([P, QB], F32, tag="misc")
            for kc in range(KC):
                nc.tensor.matmul(
                    psq[:, 0:cs],
                    w_sb[:, kc, :],
                    src_sb[:, kc, sl],
                    start=(kc == 0),
                    stop=(kc == KC - 1),
                )
            if b_sb is None:
                emit_copy(eng_of("qk_copy"), dst[:, sl], psq[:, 0:cs])
            else:
                nc.vector.tensor_scalar(
                    out=dst[:, sl], in0=psq[:, 0:cs], scalar1=b_sb[:, :],
                    scalar2=None, op0=mybir.AluOpType.add,
                )

        def emit_v_proj(kt):
            psv = ps_mi.tile([P, P], F32, tag="misc")
            for kc in range(KC):
                nc.tensor.matmul(
                    psv,
                    vt_sb[:, kc, bass.ts(kt, P)],
                    wvt_sb[:, kc, :],
                    start=(kc == 0),
                    stop=(kc == KC - 1),
                )
            dst = v_aug[kt][:, :, 0:DK]
            src = psv[:, :].rearrange("p (h f) -> p h f", h=2)
            if with_bias:
                nc.vector.scalar_tensor_tensor(
                    out=dst, in0=src, scalar=1.0,
                    in1=bv_sb[:, :].rearrange("p (h f) -> p h f", h=2),
                    op0=mybir.AluOpType.mult, op1=mybir.AluOpType.add,
                )
            else:
                emit_copy(eng_of("v_copy"), dst, src)

        def emit_qk_exp(qb, kt):
            """QK + exp for one key tile.  ACT tiles: exact exp.  DVE tiles
            (separate 1-buf score tag): phase-averaged Schraudolph, 3 DVE ops
            approximating e^s to +-0.75%."""
            ex = exp_pool.tile([P, 2, QB], BF16, tag="exps")
            dve = CFG["exp_engines"][kt] == "D"
            if not dve:
                pss = ps_s.tile([P, 2, QB], F32, tag="scores", name=f"pss{kt}")
                for h in range(2):
                    nc.tensor.matmul(
                        pss[:, h, :],
                        kT[h * DK : (h + 1) * DK, bass.ts(kt, P)],
                        qT[h * DK : (h + 1) * DK, bass.ts(qb, QB)],
                        start=True,
                        stop=True,
                    )
                nc.scalar.activation(
                    out=ex, in_=pss, func=mybir.ActivationFunctionType.Exp,
                    scale=inv_sqrt_dk,
                )
                return ex
            for h in range(2):
                pss = ps_sd.tile([P, QB], F32, tag="scoresD", name=f"pssd{kt}_{h}")
                nc.tensor.matmul(
                    pss,
                    kT[h * DK : (h + 1) * DK, bass.ts(kt, P)],
                    qT[h * DK : (h + 1) * DK, bass.ts(qb, QB)],
                    start=True,
                    stop=True,
                )
                scr1 = scr_pool.tile([P, QB], BF16, tag="scr1", name="scr1")
                scr2 = scr_pool.tile([P, QB], BF16, tag="scr2", name="scr2")
                nc.vector.tensor_scalar(
                    out=scr1.bitcast(U16), in0=pss,
                    scalar1=16.0 * LOG2E, scalar2=B16_SCH1,
                    op0=mybir.AluOpType.mult, op1=mybir.AluOpType.add,
                )
                nc.vector.tensor_scalar(
                    out=scr2.bitcast(U16), in0=pss,
                    scalar1=16.0 * LOG2E, scalar2=B16_SCH2,
                    op0=mybir.AluOpType.mult, op1=mybir.AluOpType.add,
                )
                nc.vector.scalar_tensor_tensor(
                    out=ex[:, h, :], in0=scr2, scalar=C_SCH, in1=scr1,
                    op0=mybir.AluOpType.mult, op1=mybir.AluOpType.add,
                )
            return ex

        def tail_pieces(qb, exs):
            """Yield tail emission pieces for one query block, interleaved
            between the next block's QK+exp emissions.  The LAST block's tail
            borrows the then-idle scores psum ring for 2-deep pipelining."""
            last = qb == NQB - 1
            pv_pool = ps_s if last else ps_pv
            mi_pool = ps_mi
            ptag = "scores" if last else "pv"
            mtag = "misc"
            for qs in range(QSUB):
                # one pv BANK per qsub: head A accumulates + is divided out,
                # then head B reuses the same bank (A's data fully read first)
                st = {}

                def emit_pv(h, k0, k1, qs=qs, st=st, exs=exs):
                    if h == 0 and k0 == 0:
                        st["pv"] = pv_pool.tile(
                            [P, VW], F32, tag=ptag, name=f"pv{qb}_{qs}"
                        )
                        st["at"] = att_pool.tile([P, 2, DK], BF16, tag="att",
                                                 name="at")
                    pv = st["pv"]
                    for kt in range(k0, k1):
                        nc.tensor.matmul(
                            pv,
                            exs[kt][:, h, bass.ts(qs, P)],
                            v_aug[kt][:, h, :],
                            start=(kt == 0),
                            stop=(kt == NKT - 1),
                        )

                def emit_div(h, qs=qs, st=st):
                    rden = att_pool.tile([P, 1], F32, tag="rden", name="rden")
                    nc.vector.reciprocal(rden, st["pv"][:, DK : DK + 1])
                    nc.vector.tensor_scalar(
                        out=st["at"][:, h, :],
                        in0=st["pv"][:, 0:DK],
                        scalar1=rden[:, :],
                        scalar2=None,
                        op0=mybir.AluOpType.mult,
                    )

                def emit_tp(qs=qs, st=st):
                    att_tp = (ps_sd if last else ps_mi).tile(
                        [P, P], BF16, tag="scoresD" if last else "misc",
                        name="att_tp")
                    nc.tensor.transpose(
                        att_tp, st["at"].rearrange("p a b -> p (a b)"), ident_sb
                    )
                    att_t = att_pool.tile([P, P], BF16, tag="attT", name="att_t")
                    emit_copy(eng_of("attT_copy"), att_t, att_tp)
                    st["att_t"] = att_t

                def emit_wo(qs=qs, st=st):
                    pso = mi_pool.tile([P, D], F32, tag=mtag, name="pso")
                    nc.tensor.matmul(pso, st["att_t"], wot_sb, start=True, stop=True)
                    out_sb = out_pool.tile([P, D], F32, tag="outs", name="out_sb")
                    emit_copy(eng_of("out_copy"), out_sb, pso)
                    nc.sync.dma_start(
                        out=out_d[bass.ds(qb * QB + qs * P, P), :],
                        in_=out_sb,
                    )

                yield lambda: emit_pv(0, 0, NKT)
                yield lambda: emit_div(0)
                yield lambda: emit_pv(1, 0, NKT)
                yield lambda: (emit_div(1), emit_tp())
                yield emit_wo

        # ---- main schedule ----
        bqs = bq_sb if with_bias else None
        bks = bk_sb if with_bias else None
        kpc = NKT // NTC  # key tiles per chunk
        exs0 = []
        emit_qk_proj(kT, kt_sb, wkt_sb, bks, 0, lo=0, cs=P)
        emit_qk_proj(qT, qt_sb, wqt_sb, bqs, 0, lo=0, cs=T // NTC // 2)
        emit_qk_proj(qT, qt_sb, wqt_sb, bqs, 0, lo=T // NTC // 2, cs=T // NTC // 2)
        exs0.append(emit_qk_exp(0, 0))
        emit_qk_proj(kT, kt_sb, wkt_sb, bks, 0, lo=P, cs=T // NTC - P)
        for c in range(NTC):
            if c + 1 < NTC:
                emit_qk_proj(kT, kt_sb, wkt_sb, bks, c + 1)
            if c == 0:
                for kt in range(kpc):
                    emit_v_proj(kt)
            for kt in range(max(1, c * kpc), (c + 1) * kpc):
                exs0.append(emit_qk_exp(0, kt))
            if c + 1 < NTC:
                for kt in range((c + 1) * kpc, (c + 2) * kpc):
                    emit_v_proj(kt)
                emit_qk_proj(qT, qt_sb, wqt_sb, bqs, c + 1)
        prev = (0, exs0)
        for qb in range(1, NQB):
            pieces = tail_pieces(*prev)
            exs = []
            for kt in range(NKT):
                exs.append(emit_qk_exp(qb, kt))
                for _ in range(2 if kt % 4 == 0 else 1):
                    piece = next(pieces, None)
                    if piece is not None:
                        piece()
            for piece in pieces:
                piece()
            prev = (qb, exs)
        for piece in tail_pieces(*prev):
            piece()

    nc.compile()
    return nc


_NC_CACHE = {}


def _get_nc(with_bias):
    if with_bias not in _NC_CACHE:
        _NC_CACHE[with_bias] = _build_bass(with_bias)
    return _NC_CACHE[with_bias]


def _prep_in_maps(Q, K, V, Wq, bq, Wk, bk, Wv, bv, Wo, bo, with_bias):
    bf = ml_dtypes.bfloat16
    f32 = np.float32
    qkvT = []
    for X in (Q, K, V):
        qkvT.append([np.ascontiguousarray(X[b].T.astype(bf)) for b in range(B)])
    woT = np.ascontiguousarray(Wo.T.astype(bf))

    def swz(w_rows):
        return np.ascontiguousarray(
            w_rows.T.astype(bf).reshape(KC, P, P).transpose(1, 0, 2)
        )

    in_maps = []
    for c in range(N_CORES):
        b, p = divmod(c, 4)
        rows = slice(P * p, P * (p + 1))
        m = {
            "qt": qkvT[0][b],
            "ktin": qkvT[1][b],
            "vt": qkvT[2][b],
            "wqkv": np.ascontiguousarray(
                np.stack([swz(Wq[rows]), swz(Wk[rows]), swz(Wv[rows])], axis=1)
            ),
            "wot": np.ascontiguousarray(woT[rows]),
        }
        if with_bias:
            m["bq"] = np.ascontiguousarray(bq[rows].astype(f32).reshape(P, 1))
            m["bk"] = np.ascontiguousarray(bk[rows].astype(f32).reshape(P, 1))
            m["bv"] = np.ascontiguousarray(bv[rows].astype(f32).reshape(1, P))
        in_maps.append(m)
    return in_maps


def kernel(Q, K, V, Wq, bq, Wk, bk, Wv, bv, Wo, bo, _return_raw=False):
    Q, K, V = np.asarray(Q), np.asarray(K), np.asarray(V)
    Wq, Wk, Wv, Wo = (np.asarray(x) for x in (Wq, Wk, Wv, Wo))
    bq, bk, bv, bo = (np.asarray(x) for x in (bq, bk, bv, bo))
    with_bias = bool(np.any(bq) or np.any(bk) or np.any(bv))
    nc = _get_nc(with_bias)
    in_maps = _prep_in_maps(Q, K, V, Wq, bq, Wk, bk, Wv, bv, Wo, bo, with_bias)
    try:
        res = run_bass_kernel_spmd(
            nc,
            in_maps,
            core_ids=list(range(N_CORES)),
            trace=os.environ.get("KERNEL_TRACE", "0") == "1",
        )
    except ModuleNotFoundError:
        os.environ["BASS_NEVER_TRACE"] = "1"
        res = run_bass_kernel_spmd(
            nc, in_maps, core_ids=list(range(N_CORES)), trace=False
        )
    parts = [r["outp"] for r in res.results]
    out = np.empty((B, T, D), np.float32)
    for b in range(B):
        out[b] = parts[4 * b]
        for p in range(1, 4):
            out[b] += parts[4 * b + p]
        out[b] += bo.astype(np.float32)
    if _return_raw:
        return out, res
    return out
